# revision 2
# baseline (speedup 1.0000x reference)
"""Trainium2 Bass kernel for nn_LoraLinear (DoRA-style LoraLinear forward).

Reference computation:
    Wc   = weight + 2.0 * (lora_B @ lora_A)            # [OUT, IN]
    norm = ||Wc||_2 over OUT axis + 1e-6               # [1, IN]
    out  = x @ (lora_M * Wc / norm).T + bias           # [B, S, OUT]

Key algebraic identity used: (lora_M * Wc / norm).T applied to x equals
(x * s) @ Wc.T with s = lora_M / norm, so W_eff is never materialized.

Sharding (8 NeuronCores, tensor/column parallel):
    - OUT = 11008 = 8 * 1376 -> each core owns 1376 output columns of
      weight / lora_B / bias.
    - x, lora_A, lora_M replicated.
    - Each core computes partial column sums-of-squares of Wc over its
      OUT shard; the 8 partials are AllReduce'd (two chained collectives
      so the first one overlaps the tail of the weight stream).

Device layout: everything keyed on IN-on-partitions. Host passes W and
x pre-transposed so tiles land as [IN-part, OUT-free] / [IN-part, tok],
which is what both the TensorEngine matmuls and the free-dim norm
reduction want. Matmuls run in float32r (full-rate fp32 on the PE).
"""

import functools

import numpy as np

import concourse.tile as tile
from concourse import bacc, mybir
from concourse.bass_utils import run_bass_kernel_spmd

F32 = mybir.dt.float32
F32R = mybir.dt.float32r

NCORES = 8
B, S, IN, OUT, R = 8, 32, 4096, 11008, 64
TOK = B * S                      # 256
OSH = OUT // NCORES              # 1376 per-core output shard
NCHUNK = IN // 128               # 32 IN chunks of 128
N_TILES = (512, 512, 352)        # OUT-shard tiling for psum (<=512 fp32)
N_OFFS = (0, 512, 1024)
SCALING = 2.0
EPS = 1e-6
AR_SPLIT = 16                    # chunks in first AllReduce batch


@functools.lru_cache(maxsize=1)
def _build():
    nc = bacc.Bacc("TRN2", target_bir_lowering=False, debug=False,
                   num_devices=NCORES)

    wt = nc.dram_tensor("wt", [IN, OSH], F32R, kind="ExternalInput").ap()
    xt = nc.dram_tensor("xt", [IN, TOK], F32R, kind="ExternalInput").ap()
    ap_ = nc.dram_tensor("ap", [128, IN // 2], F32R, kind="ExternalInput").ap()
    b2t = nc.dram_tensor("b2t", [2 * R, OSH], F32R, kind="ExternalInput").ap()
    bias = nc.dram_tensor("bias", [1, OSH], F32R, kind="ExternalInput").ap()
    ones = nc.dram_tensor("ones", [1, 128], F32R, kind="ExternalInput").ap()
    mt = nc.dram_tensor("mt", [128, NCHUNK], F32, kind="ExternalInput").ap()

    out = nc.dram_tensor("out", [TOK, OSH], F32, kind="ExternalOutput").ap()

    halves = [list(range(0, AR_SPLIT)), list(range(AR_SPLIT, NCHUNK))]

    with tile.TileContext(nc) as tc:
        with (
            tc.tile_pool(name="wc", bufs=1) as wcp,
            tc.tile_pool(name="sb", bufs=1) as sb,
            tc.tile_pool(name="xs", bufs=4) as xsp,
            tc.tile_pool(name="stage", bufs=2) as stp,
            tc.tile_pool(name="ps", bufs=2, space="PSUM") as psp,
            tc.tile_pool(name="pso", bufs=1, space="PSUM") as pso,
            tc.tile_pool(name="dram", bufs=1, space="DRAM") as dram,
        ):
            # ---- constants / small tensors ----
            a_sb = sb.tile([128, IN // 2], F32R, name="a_sb")
            b2t_sb = sb.tile([2 * R, OSH], F32R, name="b2t_sb")
            bias_sb = sb.tile([1, OSH], F32R, name="bias_sb")
            ones_sb = sb.tile([1, 128], F32R, name="ones_sb")
            m_sb = sb.tile([128, NCHUNK], F32, name="m_sb")
            parts = sb.tile([128, NCHUNK * 3], F32, name="parts")

            nc.sync.dma_start(a_sb[:], ap_)
            nc.sync.dma_start(b2t_sb[:], b2t)
            nc.sync.dma_start(bias_sb[:], bias)
            nc.sync.dma_start(ones_sb[:], ones)
            nc.sync.dma_start(m_sb[:], mt)

            # ---- persistent psum accumulators for the output ----
            psum_out = {}
            for m in range(2):
                for n in range(3):
                    psum_out[m, n] = pso.tile(
                        [128, N_TILES[n]], F32, name=f"po{m}{n}", tag=f"po{m}{n}"
                    )
                    nc.tensor.matmul(
                        psum_out[m, n][:],
                        ones_sb[0:1, 0:128],
                        bias_sb[0:1, N_OFFS[n]:N_OFFS[n] + N_TILES[n]],
                        start=True, stop=False,
                    )

            # ---- phase 1: stream W, build Wc resident, accumulate norms ----
            wc_tiles = []
            for c in range(NCHUNK):
                wct = wcp.tile([128, OSH], F32R, name=f"wc{c}", tag=f"wc{c}")
                wc_tiles.append(wct)
                nc.sync.dma_start(wct[:], wt[c * 128:(c + 1) * 128, :])
                # lhsT = A chunk [64, 128] (packed layout: two IN halves
                # stacked on partitions 0-63 / 64-127)
                if c < NCHUNK // 2:
                    a_chunk = a_sb[0:64, c * 128:(c + 1) * 128]
                    rlo = 0
                else:
                    cc = c - NCHUNK // 2
                    a_chunk = a_sb[64:128, cc * 128:(cc + 1) * 128]
                    rlo = 64
                for n in range(3):
                    nsl = slice(N_OFFS[n], N_OFFS[n] + N_TILES[n])
                    pl = psp.tile([128, 512], F32, name=f"pl{c}{n}", tag="pl")
                    nc.tensor.matmul(
                        pl[:, 0:N_TILES[n]], a_chunk,
                        b2t_sb[rlo:rlo + 64, nsl],
                        start=True, stop=True,
                    )
                    # Wc = W + 2*B@A (in place over the streamed W tile)
                    nc.vector.tensor_tensor(
                        out=wct[:, nsl], in0=wct[:, nsl],
                        in1=pl[:, 0:N_TILES[n]], op=mybir.AluOpType.add,
                    )
                    # column sums of squares (free-dim accumulate on ACT);
                    # the square values land in the dead lora psum tile
                    nc.scalar.activation(
                        pl[:, 0:N_TILES[n]], wct[:, nsl],
                        mybir.ActivationFunctionType.Square,
                        accum_out=parts[:, c * 3 + n:c * 3 + n + 1],
                    )

            # ---- phase 2: two chained AllReduces + s = m/(sqrt(n2)+eps) ----
            s_half = {}
            for h, chunks in enumerate(halves):
                nch = len(chunks)
                lo = chunks[0]
                n2 = sb.tile([128, nch], F32, name=f"n2_{h}", tag=f"n2_{h}")
                nc.vector.reduce_sum(
                    n2[:],
                    parts[:, lo * 3:(lo + nch) * 3].rearrange(
                        "p (c o) -> p c o", o=3),
                    axis=mybir.AxisListType.X,
                )
                cin = dram.tile([128, nch], F32, name=f"ci{h}", tag=f"ci{h}")
                cout = dram.tile([128, nch], F32, name=f"co{h}", tag=f"co{h}",
                                 addr_space="Shared")
                nc.sync.dma_start(cin[:], n2[:])
                nc.gpsimd.collective_compute(
                    "AllReduce", mybir.AluOpType.add,
                    replica_groups=[list(range(NCORES))],
                    ins=[cin[:].opt()], outs=[cout[:].opt()],
                )
                n2g = sb.tile([128, nch], F32, name=f"n2g_{h}", tag=f"n2g_{h}")
                nc.sync.dma_start(n2g[:], cout[:])

                # s = m / (sqrt(n2g) + eps); ACT sqrt refined with one
                # Newton step, DVE reciprocal refined likewise
                y0 = sb.tile([128, nch], F32, name=f"y0_{h}", tag=f"y0_{h}")
                nc.scalar.activation(y0[:], n2g[:],
                                     mybir.ActivationFunctionType.Sqrt)
                r0 = sb.tile([128, nch], F32, name=f"r0_{h}", tag=f"r0_{h}")
                nc.vector.reciprocal(r0[:], y0[:])
                t0 = sb.tile([128, nch], F32, name=f"t0_{h}", tag=f"t0_{h}")
                nc.vector.tensor_tensor(out=t0[:], in0=n2g[:], in1=r0[:],
                                        op=mybir.AluOpType.mult)
                y1 = sb.tile([128, nch], F32, name=f"y1_{h}", tag=f"y1_{h}")
                nc.vector.tensor_tensor(out=y1[:], in0=y0[:], in1=t0[:],
                                        op=mybir.AluOpType.add)
                nc.vector.tensor_scalar(out=y1[:], in0=y1[:], scalar1=0.5,
                                        scalar2=EPS, op0=mybir.AluOpType.mult,
                                        op1=mybir.AluOpType.add)
                r1 = sb.tile([128, nch], F32, name=f"r1_{h}", tag=f"r1_{h}")
                nc.vector.reciprocal(r1[:], y1[:])
                t2 = sb.tile([128, nch], F32, name=f"t2_{h}", tag=f"t2_{h}")
                nc.vector.tensor_tensor(out=t2[:], in0=y1[:], in1=r1[:],
                                        op=mybir.AluOpType.mult)
                # r2 = r1*(2 - t2) = r1 + r1*(1-t2); use tensor_scalar then mult
                u = sb.tile([128, nch], F32, name=f"u_{h}", tag=f"u_{h}")
                nc.vector.tensor_scalar(out=u[:], in0=t2[:], scalar1=-1.0,
                                        scalar2=2.0, op0=mybir.AluOpType.mult,
                                        op1=mybir.AluOpType.add)
                r2 = sb.tile([128, nch], F32, name=f"r2_{h}", tag=f"r2_{h}")
                nc.vector.tensor_tensor(out=r2[:], in0=r1[:], in1=u[:],
                                        op=mybir.AluOpType.mult)
                sh = sb.tile([128, nch], F32, name=f"s_{h}", tag=f"s_{h}")
                nc.vector.tensor_tensor(out=sh[:], in0=m_sb[:, lo:lo + nch],
                                        in1=r2[:], op=mybir.AluOpType.mult)
                s_half[h] = sh

            # ---- phase 3: xs = xt * s, main matmuls accumulate ----
            for h, chunks in enumerate(halves):
                sh = s_half[h]
                for j, c in enumerate(chunks):
                    xtile = xsp.tile([128, TOK], F32R, name=f"x{c}", tag="x")
                    nc.sync.dma_start(xtile[:], xt[c * 128:(c + 1) * 128, :])
                    nc.vector.tensor_scalar_mul(
                        xtile[:], xtile[:], sh[:, j:j + 1])
                    last = c == NCHUNK - 1
                    for m in range(2):
                        lhs = xtile[:, m * 128:(m + 1) * 128]
                        for n in range(3):
                            nsl = slice(N_OFFS[n], N_OFFS[n] + N_TILES[n])
                            nc.tensor.matmul(
                                psum_out[m, n][:], lhs, wc_tiles[c][:, nsl],
                                start=False, stop=last,
                            )

            # ---- phase 4: copy psum -> sbuf -> HBM ----
            for m in range(2):
                for n in range(3):
                    st = stp.tile([128, N_TILES[n]], F32, name=f"st{m}{n}",
                                  tag="st")
                    nc.vector.tensor_copy(st[:], psum_out[m, n][:])
                    nc.sync.dma_start(
                        out[m * 128:(m + 1) * 128,
                            N_OFFS[n]:N_OFFS[n] + N_TILES[n]],
                        st[:],
                    )

    nc.compile()
    return nc


def _prep_inputs(x, weight, lora_A, lora_B, lora_M, bias):
    """Shard + lay out the full inputs for the 8 cores (host-side data
    marshaling only)."""
    x = np.ascontiguousarray(np.asarray(x, np.float32))
    weight = np.asarray(weight, np.float32)
    lora_A = np.asarray(lora_A, np.float32)
    lora_B = np.asarray(lora_B, np.float32)
    lora_M = np.asarray(lora_M, np.float32)
    bias = np.asarray(bias, np.float32)

    xt = np.ascontiguousarray(x.reshape(TOK, IN).T)              # [IN, TOK]
    a_packed = np.empty((128, IN // 2), np.float32)
    a_packed[0:64] = lora_A[:, 0:IN // 2]
    a_packed[64:128] = lora_A[:, IN // 2:]
    mt = np.ascontiguousarray(lora_M.reshape(NCHUNK, 128).T)     # [128, 32]
    ones = np.ones((1, 128), np.float32)

    in_maps = []
    for c in range(NCORES):
        sl = slice(c * OSH, (c + 1) * OSH)
        in_maps.append(dict(
            wt=np.ascontiguousarray(weight[sl, :].T),            # [IN, OSH]
            xt=xt,
            ap=a_packed,
            b2t=np.ascontiguousarray(np.concatenate(
                [(SCALING * lora_B[sl, :]).T] * 2, axis=0)),     # [2R, OSH]
            bias=np.ascontiguousarray(bias[sl].reshape(1, OSH)),
            ones=ones,
            mt=mt,
        ))
    return in_maps


def _run(inputs, trace=False):
    nc = _build()
    in_maps = _prep_inputs(**inputs)
    res = run_bass_kernel_spmd(nc, in_maps, core_ids=list(range(NCORES)),
                               trace=trace)
    full = np.concatenate([res.results[c]["out"] for c in range(NCORES)],
                          axis=1)
    return full.reshape(B, S, OUT), res


def kernel(x, weight, lora_A, lora_B, lora_M, bias):
    out, _ = _run(dict(x=x, weight=weight, lora_A=lora_A, lora_B=lora_B,
                       lora_M=lora_M, bias=bias))
    return out


def kernel_profiled(**inputs):
    """Like kernel() but with NTFF tracing; returns (out, exec_time_ns)."""
    out, res = _run(inputs, trace=True)
    return out, res.exec_time_ns
